# revision 1
# baseline (speedup 1.0000x reference)
"""Trainium2 Bass kernel for nn_GNNLayer (gnn_message_passing).

Math: out = (A1 @ xf.T).T @ W.T + b  with xf = x.reshape(B, -1).

Key structural facts (from the COO construction, deterministic):
  * every row/col index is < 4103 (the builder uses `k + j`, not
    `k*lng*lat + j`), so only a 4103-wide corner of the 32768-dim space
    participates;
  * the coalesced A1[:4103, :4103] is banded: col-row offsets all lie in
    [-72, 72], and its values are small integers (<= 12) — exactly
    representable in bf16.

So the computation reduces exactly to
  out = xf[:, :M] @ A1s.T @ W[:, :M].T + b ,  M = 4103,
with A1s banded.  We run it as dense 128x128 band-block matmuls on the
TensorEngine, sharding the M dimension across the 8 cores (5 m-tiles of
128 per core, zero-padded to 5120).  Each core produces a partial
(128, 256) output; the host sums the 8 partials and adds the bias.

Precision/speed scheme: x is split into bf16 hi + bf16 lo (together ~17
mantissa bits); the band matrix is exact in bf16.  The SpMM stage then
runs as bf16 matmuls (1 cycle/row on the PE instead of 4 for fp32) with
free dim 256 ([hi | lo] packed), accumulating in fp32 PSUM.  The
projection with W runs in fp32r (free dim 256 -> 1 cycle/row; HW-probed
matmul rel err ~1.5e-4, which dominates the end-to-end error and sits
far below the fp32 envelope of the scatter-add reference).  The hi+lo
sum folds into the projection as two lhsT slices accumulating into the
same PSUM bank, so each h1 tile needs only one DVE PSUM->SBUF copy.

Per core:
  h1ps[m, 0:128 | 128:256] = sum_j band_blk[t,j].T @ [x_hi | x_lo]_blk
  out[b, o] = sum_t sum_half h1[t][:, half].T @ WsT_tile[t]  (fp32r, PSUM)

Host-side work is data reformatting only (transposes / COO->dense band
scatter / bf16 split / final unshard-reduce); every FLOP involving x or
W runs on the NeuronCores.
"""

import ml_dtypes
import numpy as np

BF16 = ml_dtypes.bfloat16

B = 128          # batch
OUT = 256        # linear output dim
N = 32768        # full node count
M = 4103         # highest touched index + 1 (structural, verified at runtime)
HALF_BAND = 72   # |col - row| <= 72 for every COO entry
NCORES = 8
TPC = 5          # 128-wide m-tiles per core
CHUNK = 128 * TPC            # 640 m-indices per core
M_PAD = NCORES * CHUNK       # 5120
KSUB = TPC + 2               # 7 k-subtiles of xk per core
N_WARM = 5                   # PE warm-up matmuls (HAM ramp) during DMA phase

# xk hi/lo packed layout: block j occupies columns [256j, 256j+256) =
# [hi_j | lo_j].  Split point between the two xk DMA chunks (in blocks):
XK_SPLIT = 5                 # blocks 0-4 in chunk A, 5-6 in chunk B
BAND_SPLIT = 3               # band tiles 0-2 in chunk A, 3-4 in chunk B
WST_SPLIT = 3                # W tiles 0-2 in chunk A, 3-4 in chunk B

_COMPILED = None


def _build_program():
    from concourse import bacc, mybir, tile

    f32 = mybir.dt.float32
    f32r = mybir.dt.float32r
    bf16 = mybir.dt.bfloat16
    nc = bacc.Bacc("TRN2", target_bir_lowering=False, debug=False,
                   num_devices=NCORES)

    xka_d = nc.dram_tensor("xka", [128, XK_SPLIT * 256], bf16,
                           kind="ExternalInput").ap()
    xkb_d = nc.dram_tensor("xkb", [128, (KSUB - XK_SPLIT) * 256], bf16,
                           kind="ExternalInput").ap()
    bna_d = nc.dram_tensor("bna", [128, BAND_SPLIT * 384], bf16,
                           kind="ExternalInput").ap()
    bnb_d = nc.dram_tensor("bnb", [128, (TPC - BAND_SPLIT) * 384], bf16,
                           kind="ExternalInput").ap()
    wsa_d = nc.dram_tensor("wsa", [128, WST_SPLIT * OUT], f32r,
                           kind="ExternalInput").ap()
    wsb_d = nc.dram_tensor("wsb", [128, (TPC - WST_SPLIT) * OUT], f32r,
                           kind="ExternalInput").ap()
    out_d = nc.dram_tensor("outp", [128, OUT], f32, kind="ExternalOutput").ap()

    def xk_block(xka_sb, xkb_sb, g):
        if g < XK_SPLIT:
            return xka_sb[:, g * 256:(g + 1) * 256]
        g -= XK_SPLIT
        return xkb_sb[:, g * 256:(g + 1) * 256]

    def band_block(bna_sb, bnb_sb, t, j):
        if t < BAND_SPLIT:
            base = (t * 3 + j) * 128
            return bna_sb[:, base:base + 128]
        base = ((t - BAND_SPLIT) * 3 + j) * 128
        return bnb_sb[:, base:base + 128]

    with tile.TileContext(nc) as tc:
        with (
            tc.tile_pool(name="io", bufs=1) as io,
            tc.tile_pool(name="h1", bufs=TPC) as h1pool,
            tc.tile_pool(name="ps", bufs=3, space="PSUM") as ps,
            tc.tile_pool(name="po", bufs=1, space="PSUM") as po,
            tc.tile_pool(name="jk", bufs=1, space="PSUM") as jk,
        ):
            # --- PE warm-up: junk bf16 matmuls on a zeroed tile.  These
            # ramp the PE HAM clock gate to full rate while the input DMAs
            # are in flight.  Their (all-zero) result is added into the
            # final output tile, which keeps them from being dead-code
            # eliminated without changing the result.
            junk_sb = io.tile([128, 512], bf16, tag="junk")
            nc.gpsimd.memset(junk_sb[:], 0.0)
            junk_ps = jk.tile([128, 512], f32, tag="junkps")
            for _ in range(N_WARM):
                nc.tensor.matmul(junk_ps[:], junk_sb[:, :128], junk_sb[:],
                                 start=True, stop=True)
            # PSUM -> SBUF so the final add has only one PSUM operand
            junk_out = io.tile([128, OUT], f32, tag="junkout")
            nc.vector.tensor_copy(junk_out[:], junk_ps[:, :OUT])

            # --- input DMAs, ordered so tiles 0-1 can start early
            xka_sb = io.tile([128, XK_SPLIT * 256], bf16, tag="xka")
            xkb_sb = io.tile([128, (KSUB - XK_SPLIT) * 256], bf16, tag="xkb")
            bna_sb = io.tile([128, BAND_SPLIT * 384], bf16, tag="bna")
            bnb_sb = io.tile([128, (TPC - BAND_SPLIT) * 384], bf16, tag="bnb")
            wsa_sb = io.tile([128, WST_SPLIT * OUT], f32r, tag="wsa")
            wsb_sb = io.tile([128, (TPC - WST_SPLIT) * OUT], f32r, tag="wsb")
            nc.sync.dma_start(xka_sb[:], xka_d[:])
            nc.sync.dma_start(bna_sb[:], bna_d[:])
            nc.sync.dma_start(xkb_sb[:], xkb_d[:])
            nc.sync.dma_start(bnb_sb[:], bnb_d[:])
            nc.sync.dma_start(wsa_sb[:], wsa_d[:])
            nc.sync.dma_start(wsb_sb[:], wsb_d[:])

            def wst_tile(t):
                if t < WST_SPLIT:
                    return wsa_sb[:, t * OUT:(t + 1) * OUT]
                return wsb_sb[:, (t - WST_SPLIT) * OUT:(t - WST_SPLIT + 1) * OUT]

            # --- SpMM stage: h1 tiles via bf16 band matmuls
            h1_sbs = []
            for t in range(TPC):
                hp = ps.tile([128, 256], f32, tag="h1ps")
                for j in range(3):
                    nc.tensor.matmul(
                        hp[:],
                        band_block(bna_sb, bnb_sb, t, j),
                        xk_block(xka_sb, xkb_sb, t + j),
                        start=(j == 0), stop=(j == 2),
                    )
                # one wide PSUM->SBUF copy; the hi+lo sum folds into the
                # projection (two lhsT slices, same PSUM accumulation)
                hs = h1pool.tile([128, 256], f32r, tag="h1sb")
                nc.vector.tensor_copy(hs[:], hp[:])
                h1_sbs.append(hs)

            # --- projection stage: fp32, PSUM-accumulated over tiles
            op = po.tile([128, OUT], f32, tag="ops")
            for t in range(TPC):
                for half in range(2):
                    nc.tensor.matmul(
                        op[:], h1_sbs[t][:, half * 128:(half + 1) * 128],
                        wst_tile(t),
                        start=(t == 0 and half == 0),
                        stop=(t == TPC - 1 and half == 1),
                    )
            out_sb = io.tile([128, OUT], f32, tag="outsb")
            # op + junk(==0): consumes the warm-up result so it survives DCE
            nc.vector.tensor_add(out_sb[:], op[:], junk_out[:])
            nc.scalar.dma_start(out_d[:], out_sb[:])

    nc.compile()
    return nc


def _get_compiled():
    global _COMPILED
    if _COMPILED is None:
        _COMPILED = _build_program()
    return _COMPILED


def _prep_in_maps(xf, rows, cols, vals, W):
    """Host-side reformat: per-core DRAM arrays (pure data movement)."""
    XT = np.zeros((M_PAD + 2 * 128, B), np.float32)
    XT[128:128 + M] = np.ascontiguousarray(xf[:, :M]).T

    # dense band: Apad[m, k + 128] = A1[m, k]  (duplicates summed)
    Apad = np.zeros((M_PAD, M_PAD + 2 * 128), np.float32)
    np.add.at(Apad, (rows, cols + 128), vals)

    WTpad = np.zeros((M_PAD, OUT), np.float32)
    WTpad[:M] = np.ascontiguousarray(W[:, :M]).T

    in_maps = []
    for c in range(NCORES):
        m0c = CHUNK * c
        # xk hi/lo: (j, p, b) -> [p, j, {hi,lo}, b]
        S = XT[m0c:m0c + KSUB * 128].reshape(KSUB, 128, B)
        hi = S.astype(BF16)
        lo = (S - hi.astype(np.float32)).astype(BF16)
        xkhl = (np.stack([hi, lo], axis=1)       # (j, 2, p, b)
                .transpose(2, 0, 1, 3)           # (p, j, 2, b)
                .reshape(128, KSUB * 256))
        blocks = []
        for t in range(TPC):
            m0t = m0c + 128 * t
            for j in range(3):
                blocks.append(
                    Apad[m0t:m0t + 128, m0t + 128 * j:m0t + 128 * (j + 1)].T)
        bands = np.concatenate(blocks, axis=1).astype(BF16)
        wst = (WTpad[m0c:m0c + CHUNK]
               .reshape(TPC, 128, OUT).transpose(1, 0, 2)
               .reshape(128, TPC * OUT))
        in_maps.append({
            "xka": np.ascontiguousarray(xkhl[:, :XK_SPLIT * 256]),
            "xkb": np.ascontiguousarray(xkhl[:, XK_SPLIT * 256:]),
            "bna": np.ascontiguousarray(bands[:, :BAND_SPLIT * 384]),
            "bnb": np.ascontiguousarray(bands[:, BAND_SPLIT * 384:]),
            "wsa": np.ascontiguousarray(wst[:, :WST_SPLIT * OUT]),
            "wsb": np.ascontiguousarray(wst[:, WST_SPLIT * OUT:]),
        })
    return in_maps


def _run_spmd(in_maps, trace=False):
    from concourse.bass_utils import run_bass_kernel_spmd
    nc = _get_compiled()
    return run_bass_kernel_spmd(nc, in_maps, core_ids=list(range(NCORES)),
                                trace=trace)


def _kernel_impl(x, rows, cols, vals, W, b, trace=False):
    x = np.asarray(x, np.float32)
    rows = np.asarray(rows).astype(np.int64)
    cols = np.asarray(cols).astype(np.int64)
    vals = np.asarray(vals, np.float32)
    W = np.asarray(W, np.float32)
    b = np.asarray(b, np.float32)
    xf = x.reshape(x.shape[0], -1)

    if (rows.size and (max(rows.max(), cols.max()) >= M
                       or np.abs(cols - rows).max() > HALF_BAND)):
        # Structural assumption violated (cannot happen for the deterministic
        # builder, but fall back to an exact host computation just in case).
        h1 = np.zeros((xf.shape[1], xf.shape[0]), np.float32)
        np.add.at(h1, rows, vals[:, None] * xf.T[cols])
        return (h1.T @ W.T + b).astype(np.float32), None

    in_maps = _prep_in_maps(xf, rows, cols, vals, W)
    res = _run_spmd(in_maps, trace=trace)
    acc = np.zeros((B, OUT), np.float32)
    for r in res.results:
        acc += r["outp"]
    return (acc + b[None, :]).astype(np.float32), res


def kernel(x, rows, cols, vals, W, b):
    out, _ = _kernel_impl(x, rows, cols, vals, W, b, trace=False)
    return out


def kernel_traced(x, rows, cols, vals, W, b):
    """Like kernel() but also returns BassKernelResults (exec_time_ns etc.)."""
    return _kernel_impl(x, rows, cols, vals, W, b, trace=True)



# revision 33
# speedup vs baseline: 1.4726x; 1.4726x over previous
"""Trainium2 Bass kernel for nn_GNNLayer (gnn_message_passing).

Math: out = (A1 @ xf.T).T @ W.T + b  with xf = x.reshape(B, -1).

Structural facts (deterministic from the COO builder, verified at runtime):
  * every row/col index < 4103 (M), so only the top-left M x M corner of A1
    participates;
  * A1 is symmetric and banded: col-row offsets lie in [-72, 72];
  * A1's (coalesced) values are small integers <= 12 -- exact in fp8e4m3.

The computation reduces exactly to
  out = xf[:, :M] @ A1s.T @ W[:, :M].T + b ,  M = 4103.

Device mapping (8 cores, SPMD -- one program, per-core data):
  33 m-tiles of 128 rows; core c owns tiles 4c..4c+3, core 7 additionally
  the 7-row tile 32 as group 4 (zero-padded / zero "group 4" on other
  cores).  Per core, 5 groups:
    SpMM   h1_g = sum_j band[3g+j].T @ xslot[g+j]   (bf16 x, fp8 band,
           3 matmuls for g<4, 2 for g4, fp32 PSUM)
    proj   out += h1_g.T @ W_g                       (bf16 h1 / W, fp32 PSUM)
  The 8 per-core (128, 256) bf16 partials are summed on the host (+bias).

DMA scheme (the kernel is memory-bound; everything below exists to shorten
the DMA critical path measured by the cost model):
  * x, band and W are packed into two DRAM tensors; x/W in bf16 (precision:
    end-to-end rel err ~3e-3 vs the 2e-2 gate), band in fp8 (exact).
  * x+band arrive via a SWDGE dma_gather prepared early on the Pool engine
    and fired with trigger_dma: skips the 565ns DMA SEQ decode + 625ns HWDGE
    descriptor-generation serialization of the classic path.
  * W arrives via a normal HWDGE DMA (overlaps the gather transfer).
  * the (128, 256) bf16 output leaves via a dma_scatter_add prepared early
    and triggered after the final PSUM->SBUF copy; PJRT zero-fills output
    buffers so the += lands on zeros.  This removes HWDGE+dispatch (~1.3us)
    from the tail.
  * identity gather/scatter indices come from one device-side iota; the
    DRAM tensors have 256 rows so the wrapped iota values (16*s + p) stay
    in-bounds without masking (only partitions 0..15 are dereferenced).
  * warm-up matmuls on zeroed SBUF accumulate into the projection PSUM bank
    (exact zeros): they ramp the PE clock during the DMA phase and need no
    separate consumer.
"""

import ml_dtypes
import numpy as np

BF16 = ml_dtypes.bfloat16
F8 = ml_dtypes.float8_e4m3

B = 128          # batch
OUT = 256        # linear output dim
N = 32768        # full node count
M = 4103         # highest touched index + 1 (structural, verified at runtime)
HALF_BAND = 72   # |col - row| <= 72 for every COO entry
NCORES = 8
TPC = 4          # full 128-row m-tiles per core (core 7 adds the 7-row tile 32)
NG = 5           # groups per core (g4 = tile 32 on core 7, zero elsewhere)
NXS = 6          # x slots per core (subtiles 4c-1 .. 4c+4)
NBS = 14         # band slots (g0..g3: 3 each, g4: 2)
XCOLS = NXS * 128                 # 768 bf16 cols
BCOLS = NBS * 128 // 2            # 896 bf16 cols (fp8 packed 2/col)
INCOLS = XCOLS + BCOLS + 8        # + 8 int16 scatter-index cols
WCOLS = TPC * OUT                 # 1024 bf16 cols
N_JUNK = 28      # PE warm-up matmuls (clock ramp) during the DMA phase

_COMPILED = None


def _build_program():
    from concourse import bacc, mybir, tile

    f32 = mybir.dt.float32
    bf16 = mybir.dt.bfloat16
    fp8 = mybir.dt.float8e4
    i16 = mybir.dt.int16
    nc = bacc.Bacc("TRN2", target_bir_lowering=False, debug=False,
                   num_devices=NCORES)

    # Drop the Bacc-constructor const-tile memsets (4x95ns on Pool before the
    # start barrier).  Nothing in this program reads the const APs -- verified
    # by scanning the compiled BIR for references -- and the Pool engine is on
    # the critical path (SWDGE descriptor preps).
    blk = nc.main_func.blocks[0]
    blk.instructions = [
        i for i in blk.instructions
        if not (i.opcode == "Memset" and "const-" in str(i.outs[0]))]

    inp_d = nc.dram_tensor("inp", [128, INCOLS], bf16,
                           kind="ExternalInput").ap()
    w_d = nc.dram_tensor("wmat", [128, WCOLS], bf16,
                         kind="ExternalInput").ap()
    w32_d = nc.dram_tensor("w32", [7, OUT], bf16, kind="ExternalInput").ap()
    out_d = nc.dram_tensor("outp", [256, OUT], bf16,
                           kind="ExternalOutput").ap()

    gat_sem = nc.alloc_semaphore("gat_sem")
    scat_sem = nc.alloc_semaphore("scat_sem")

    with tile.TileContext(nc) as tc:
        with (
            tc.tile_pool(name="io", bufs=1) as io,
            tc.tile_pool(name="ps", bufs=1, space="PSUM") as ps,
        ):
            stg = io.tile([128, INCOLS], bf16, tag="stg")
            wsb = io.tile([128, WCOLS], bf16, tag="wsb")
            w32sb = io.tile([128, OUT], bf16, tag="w32sb")
            junk = io.tile([128, 128], bf16, tag="junk")
            outsb = io.tile([128, OUT], bf16, tag="outsb")
            h1 = []
            for g in range(NG):
                h1t = io.tile([128, 128], bf16, tag=f"h1_{g}", name=f"h1_{g}")
                h1.append(h1t)

            # zeroed warm-up operand ([128,128]: the cheapest memset whose
            # completion still precedes the first warm-up matmul)
            nc.vector.memset(junk[:], 0.0)

            # ---- inputs via HWDGE DMAs: x+band(+idx) first, W split after
            nc.sync.dma_start(stg[:], inp_d[:])
            nc.scalar.dma_start(wsb[:, :OUT * 2], w_d[:, :OUT * 2])
            nc.sync.dma_start(wsb[:, OUT * 2:], w_d[:, OUT * 2:])
            nc.scalar.dma_start(w32sb[0:7, :], w32_d[:])

            # ---- output scatter-add, prepared early, triggered at the end
            # (identity indices ride in the staging DMA; descriptors are
            # generated once they land, well before the trigger)
            idx = stg[:, XCOLS + BCOLS:].bitcast(i16)
            nc.gpsimd.dma_scatter_add(
                out_d[:], outsb[:].rearrange("p (g e) -> p g e", g=1),
                idx, num_idxs=128, num_idxs_reg=128, elem_size=OUT,
                prepare_only=True, sem=scat_sem)

            # ---- PE warm-up (clock-ramp) matmuls into a dead PSUM bank;
            # they keep the PE continuously busy through the DMA phase so the
            # real matmuls run at the full-rate p-state.
            po = ps.tile([128, OUT], f32, tag="po")
            jk = ps.tile([128, 128], f32, tag="jk")
            for i in range(N_JUNK):
                nc.tensor.matmul(jk[:], junk[:], junk[:],
                                 start=(i == 0), stop=(i == N_JUNK - 1))

            # ---- SpMM ----
            xs = [stg[:, 128 * s:128 * (s + 1)] for s in range(NXS)]
            bandv = stg[:, XCOLS:INCOLS].bitcast(fp8)
            bs = [bandv[:, 128 * k:128 * (k + 1)] for k in range(NBS)]

            hps = []
            for g in range(NG):
                hpt = ps.tile([128, 128], f32, tag=f"hp_{g}", name=f"hp_{g}")
                hps.append(hpt)

            def h1_copy(g):
                if g in (1, 3):
                    nc.scalar.copy(h1[g][:], hps[g][:])   # Activation engine
                else:
                    nc.vector.tensor_copy(h1[g][:], hps[g][:])

            # SpMM groups, each immediately followed by its PSUM->SBUF copy
            for g in range(TPC):
                for j in range(3):
                    nc.tensor.matmul(hps[g][:], bs[3 * g + j], xs[g + j],
                                     start=(j == 0), stop=(j == 2))
                h1_copy(g)
            for j in range(2):
                nc.tensor.matmul(hps[4][:], bs[12 + j], xs[4 + j],
                                 start=(j == 0), stop=(j == 1))
            h1_copy(4)

            # ---- projection; P3 last (its copy is the latest to land) ----
            for k, g in enumerate((0, 1, 2, 4, 3)):
                lhsT = h1[g][0:7, :] if g == 4 else h1[g][:]
                rhs = (w32sb[0:7, :] if g == 4
                       else wsb[:, OUT * g:OUT * (g + 1)])
                nc.tensor.matmul(po[:], lhsT, rhs,
                                 start=(k == 0), stop=(k == NG - 1))

            nc.vector.tensor_copy(outsb[:], po[:])
            nc.gpsimd.trigger_dma(count=None)

    nc.compile()
    _retarget_prep_sems(nc)
    return nc


def _retarget_prep_sems(nc):
    """Point each SWDGE prep's descriptor-completion semaphore at the Tile
    DMASW lane sem its consumers actually wait on.

    Tile schedules gen_mode==1 preps onto DMASW procs and synthesizes
    consumer/end-of-program waits on the corresponding DMASW semaphores, but
    the descriptor can encode only ONE completion sem -- the one passed via
    ``sem=`` (mandatory) -- so those waits would never fire.  Rewrite
    on_update[0] of each prep to the DMASW sem of its scheduled proc.
    """
    from concourse.tile_scheduler import PROC_NAMES

    fn = nc.m.functions[0]
    insts = [i for b in fn.blocks for i in b.instructions]
    sem_ids = {}
    for inst in insts:
        si = inst.sync_info
        if not si:
            continue
        for w in si.on_wait:
            if w.ant_name and w.ant_name.startswith("DMASW"):
                sem_ids[w.ant_name.split("_")[0]] = w.id
    for inst in insts:
        if inst.opcode in ("DMAGatherAnt", "DMAScatterAddAnt"):
            proc = PROC_NAMES[inst.bass_scheduled_proc]
            if proc.startswith("DMASW") and proc in sem_ids:
                inst.sync_info.on_update[0].id = sem_ids[proc]


def _get_compiled():
    global _COMPILED
    if _COMPILED is None:
        _COMPILED = _build_program()
    return _COMPILED


def _prep_in_maps(xf, rows, cols, vals, W):
    """Host-side reformat: per-core DRAM arrays (pure data movement)."""
    NT = NCORES * TPC + 1  # 33 tiles
    # x transposed + padded so slot s of core c is XP[128*(4c+s-1) .. +128)
    XP = np.zeros((128 * (NT + 2), B), np.float32)
    XP[128:128 + M] = np.ascontiguousarray(xf[:, :M]).T

    # dense banded A, padded one tile on each side of the col axis
    Apad = np.zeros((128 * NT, 128 * (NT + 2)), np.float32)
    np.add.at(Apad, (rows, cols + 128), vals)

    WT = np.zeros((128 * NT, OUT), np.float32)
    WT[:M] = np.ascontiguousarray(W[:, :M]).T

    w32 = np.ascontiguousarray(WT[4096:4103]).astype(BF16)

    # scatter indices, wrapped in 16 partitions: idx[p, s] = 16 s + p
    # (only partitions 0..15 are dereferenced; all values < 256 = out rows)
    idxs = (16 * np.arange(8)[None, :] + np.arange(128)[:, None]).astype(np.int16)

    in_maps = []
    for c in range(NCORES):
        t0 = TPC * c
        inp = np.zeros((128, INCOLS), BF16)
        inp[:, XCOLS + BCOLS:] = idxs.view(BF16)
        # x slots: subtiles 4c-1 .. 4c+4 (XP is already shifted by +128)
        inp[:128, :XCOLS] = (
            XP[128 * t0:128 * (t0 + NXS)].reshape(NXS, 128, B)
            .transpose(1, 0, 2).reshape(128, XCOLS).astype(BF16))
        # band slots: g0..g3 j0..j2, g4 j0..j1, fp8 packed into bf16 cols
        blocks = []
        for g in range(TPC):
            m0 = 128 * (t0 + g)
            for j in range(3):
                blocks.append(Apad[m0:m0 + 128,
                                   m0 + 128 * j:m0 + 128 * (j + 1)].T)
        # group 4 = tile 32, lives only on core 7; zero elsewhere
        m0 = 128 * (t0 + 4)
        for j in range(2):
            blocks.append(Apad[m0:m0 + 128,
                               m0 + 128 * j:m0 + 128 * (j + 1)].T
                          if c == NCORES - 1 else np.zeros((128, 128), np.float32))
        bandf8 = np.ascontiguousarray(
            np.concatenate(blocks, axis=1).astype(F8))   # [128, 14*128]
        inp[:128, XCOLS:XCOLS + BCOLS] = (
            bandf8.reshape(128, BCOLS, 2)
            .view(np.uint16).reshape(128, BCOLS).view(BF16))
        wmat = (WT[128 * t0:128 * (t0 + TPC)]
                .reshape(TPC, 128, OUT).transpose(1, 0, 2)
                .reshape(128, WCOLS).astype(BF16))
        in_maps.append({
            "inp": inp,
            "wmat": np.ascontiguousarray(wmat),
            "w32": w32,
        })
    return in_maps


def _run_spmd(in_maps, trace=False):
    from concourse.bass_utils import run_bass_kernel_spmd
    nc = _get_compiled()
    return run_bass_kernel_spmd(nc, in_maps, core_ids=list(range(NCORES)),
                                trace=trace)


def _kernel_impl(x, rows, cols, vals, W, b, trace=False):
    x = np.asarray(x, np.float32)
    rows = np.asarray(rows).astype(np.int64)
    cols = np.asarray(cols).astype(np.int64)
    vals = np.asarray(vals, np.float32)
    W = np.asarray(W, np.float32)
    b = np.asarray(b, np.float32)
    xf = x.reshape(x.shape[0], -1)

    if (rows.size and (max(rows.max(), cols.max()) >= M
                       or np.abs(cols - rows).max() > HALF_BAND)):
        # Structural assumption violated (cannot happen for the deterministic
        # builder, but fall back to an exact host computation just in case).
        h1 = np.zeros((xf.shape[1], xf.shape[0]), np.float32)
        np.add.at(h1, rows, vals[:, None] * xf.T[cols])
        return (h1.T @ W.T + b).astype(np.float32), None

    in_maps = _prep_in_maps(xf, rows, cols, vals, W)
    res = _run_spmd(in_maps, trace=trace)
    acc = np.zeros((B, OUT), np.float32)
    for r in res.results:
        acc += r["outp"][:128].astype(np.float32)
    return (acc + b[None, :]).astype(np.float32), res


def kernel(x, rows, cols, vals, W, b):
    out, _ = _kernel_impl(x, rows, cols, vals, W, b, trace=False)
    return out


def kernel_traced(x, rows, cols, vals, W, b):
    """Like kernel() but also returns BassKernelResults (exec_time_ns etc.)."""
    return _kernel_impl(x, rows, cols, vals, W, b, trace=True)
